# revision 2
# baseline (speedup 1.0000x reference)
"""Trainium2 Bass kernel for a 5x5 conv2d (NCHW, pad=2, stride=1), v3.

X [32,32,128,128] f32, K [64,32,5,5] f32 -> out [32,64,128,128].
Data-parallel over 8 cores, 4 images per core.

v3 trick: K=64 matmuls fuse TWO taps per instruction. SBUF holds each
image twice: partitions 64i+c = padded rows (col j = Xp[c,:,j]),
partitions 64i+32+c = the same rows shifted one column (col j =
Xp[c,:,j+1]). A [64,64] lhsT stacks taps (dy,2p) and (dy,2p+1), so one
MM with rhs col-offset 2p accumulates both. 25 taps -> 15 slots
(5 dy x [dx01, dx23, dx4+zero]). 2 images per SBUF tile -> 2 row groups
x 2 col groups = 4 64x64 PE tiles. Halves matmul+weight-load count vs
per-tap K=32 (the weight-load port is the bottleneck there).

Matmuls are chained with nosync deps to pin the issue order (the tile
scheduler otherwise serializes tile concurrency).
"""

import numpy as np

import bass_rust
import concourse.bass as bass
import concourse.tile as tile
from concourse import bacc, mybir
from concourse.bass_utils import run_bass_kernel_spmd

N_CORES = 8
IMGS = 4
C = 32
O = 64
H = W = 128
KH = KW = 5
PAD = 2
WP = W + 2 * PAD  # 132
BANDS = 4
BAND_OUT = H // BANDS         # 32
BAND_IN = BAND_OUT + 2 * PAD  # 36
RT = 4                        # rows per psum slab (RT*W = 512)
SLOTS = 15                    # 5 dy x 3 dx-pairs (last pair = dx4 + zero)
SETS = 2                      # image pairs per core

F32 = mybir.dt.float32
MM_DT = mybir.dt.bfloat16
COPY = mybir.ActivationFunctionType.Copy


def _build_nc(reps=1):
    nc = bacc.Bacc("TRN2", target_bir_lowering=False, debug=False)
    X = nc.dram_tensor("X", [IMGS, C, H, W], MM_DT, kind="ExternalInput").ap()
    # host-packed weights: [64g2+32h+c, slot, o] = K[o,c,dy,2p+h] (0 if w>4)
    K = nc.dram_tensor("K", [128, SLOTS, O], MM_DT, kind="ExternalInput").ap()
    Z = nc.dram_tensor("Z", [128, BAND_IN, 5], MM_DT, kind="ExternalInput").ap()
    ZR = nc.dram_tensor("ZR", [128, PAD, WP], MM_DT, kind="ExternalInput").ap()
    out = nc.dram_tensor("out", [IMGS, O, H, W], F32, kind="ExternalOutput").ap()

    with tile.TileContext(nc) as tc:
        with (
            tc.tile_pool(name="wpool", bufs=1) as wpool,
            tc.tile_pool(name="xpool", bufs=3) as xpool,
            tc.tile_pool(name="opool", bufs=8) as opool,
            tc.tile_pool(name="ppool", bufs=8, space="PSUM") as ppool,
        ):
            wt = wpool.tile([128, SLOTS, O], MM_DT)
            nc.sync.dma_start(wt, K)

            chain = [None]

            def mm(*args, **kwargs):
                m = nc.tensor.matmul(*args, **kwargs)
                if chain[0] is not None:
                    bass_rust.add_dep_helper(
                        m.ins, chain[0], sync=False, reason="pe-order"
                    )
                chain[0] = m.ins

            def body():
              for b in range(BANDS):
                y0 = b * BAND_OUT
                p_lo = PAD if b == 0 else 0
                p_hi = BAND_IN - 1 - PAD if b == BANDS - 1 else BAND_IN - 1
                r_lo = y0 + p_lo - PAD
                r_hi = y0 + p_hi - PAD
                for s in range(SETS):
                    xb = xpool.tile([128, BAND_IN, WP], MM_DT)
                    # zero cols [0:2] and [129:132] (covers both shifts'
                    # pad needs); data DMAs then overwrite their regions.
                    nc.sync.dma_start(xb[:, :, 0:2], Z[:, :, 0:2])
                    nc.sync.dma_start(xb[:, :, 129:132], Z[:, :, 2:5])
                    if b == 0:
                        nc.sync.dma_start(xb[:, 0:PAD, :], ZR)
                    if b == BANDS - 1:
                        nc.sync.dma_start(xb[:, BAND_IN - PAD :, :], ZR)
                    for i in range(2):
                        img = 2 * s + i
                        src = X[img, :, r_lo : r_hi + 1, :]
                        # unshifted: col j = Xp[., j]  -> data at cols 2..129
                        nc.sync.dma_start(
                            xb[64 * i : 64 * i + 32, p_lo : p_hi + 1, 2:130],
                            src,
                        )
                        # shifted: col j = Xp[., j+1] -> data at cols 1..128
                        nc.sync.dma_start(
                            xb[64 * i + 32 : 64 * i + 64, p_lo : p_hi + 1, 1:129],
                            src,
                        )

                    # one psum generation = this band x this image set:
                    # 4 waves (slab pairs), 2 banks each = 8 banks.
                    pss = [
                        [
                            ppool.tile(
                                [128, RT, W],
                                F32,
                                name=f"ps_b{b}_s{s}_w{w}_i{i}",
                                tag="ps",
                            )
                            for i in range(2)
                        ]
                        for w in range(4)
                    ]
                    for j in range(SLOTS):
                        first = j == 0
                        last = j == SLOTS - 1
                        dy, p = j // 3, j % 3
                        for w in range(4):
                            for (i, c) in ((0, 0), (1, 0), (0, 1), (1, 1)):
                                slab = 2 * w + c
                                rb = slab * RT + dy
                                mm(
                                    pss[w][i][64 * c : 64 * c + 64, :, :],
                                    wt[64 * i : 64 * i + 64, j, :],
                                    xb[
                                        64 * i : 64 * i + 64,
                                        rb : rb + RT,
                                        2 * p : 2 * p + W,
                                    ],
                                    start=first,
                                    stop=last,
                                    tile_position=(64 * i, 64 * c),
                                )
                    for w in range(4):
                        for i in range(2):
                            img = 2 * s + i
                            ob = opool.tile([128, RT, W], F32)
                            if w % 2 == 0:
                                nc.vector.tensor_copy(
                                    ob[:, :, :], pss[w][i][:, :, :]
                                )
                            else:
                                nc.scalar.activation(
                                    ob[:, :, :], pss[w][i][:, :, :], COPY
                                )
                            for c in range(2):
                                gy = y0 + (2 * w + c) * RT
                                nc.sync.dma_start(
                                    out[img, :, gy : gy + RT, :],
                                    ob[64 * c : 64 * c + 64, :, :],
                                )

            if reps > 1:
                with tc.For_i(0, reps, 1):
                    body()
            else:
                body()
    nc.compile()
    return nc


_CACHE = {}


def _get_nc(reps=1):
    if reps not in _CACHE:
        _CACHE[reps] = _build_nc(reps)
    return _CACHE[reps]


def make_in_maps(X, K):
    import ml_dtypes

    BF = ml_dtypes.bfloat16
    X = np.asarray(X, dtype=np.float32)
    K = np.asarray(K, dtype=np.float32)
    # pack weights: Kp[64g2+32h+c, 3*dy+p, o] = K[o, c, dy, 2p+h] (0 if col>4)
    Kp = np.zeros((128, SLOTS, O), dtype=np.float32)
    for dy in range(KH):
        for p in range(3):
            for h2 in range(2):
                wcol = 2 * p + h2
                if wcol >= KW:
                    continue
                blk = K[:, :, dy, wcol].T  # [c, o]
                for g2 in range(2):
                    Kp[64 * g2 + 32 * h2 : 64 * g2 + 32 * h2 + 32, 3 * dy + p] = blk
    Kp = Kp.astype(BF)
    per = X.shape[0] // N_CORES
    Z = np.zeros((128, BAND_IN, 5), dtype=BF)
    ZR = np.zeros((128, PAD, WP), dtype=BF)
    return [
        {
            "X": np.ascontiguousarray(X[per * i : per * (i + 1)]).astype(BF),
            "K": Kp,
            "Z": Z,
            "ZR": ZR,
        }
        for i in range(N_CORES)
    ]


def kernel(X, K):
    nc = _get_nc()
    in_maps = make_in_maps(X, K)
    res = run_bass_kernel_spmd(nc, in_maps, list(range(N_CORES))).results
    return np.concatenate([res[i]["out"] for i in range(N_CORES)], axis=0)
